# revision 1
# baseline (speedup 1.0000x reference)
"""Trainium2 Bass kernel for nn_CatEdgeGraphLayer.

Reference computation (B=128, N=64, D=128, OUT=128):
    f_i = af[:, :, None, :], f_j = af[:, None, :, :]
    msg = A[..., None] * cat(f_j, diff)              # [B,N,N,D+2]
    inp = cat(f_i, msg)                              # [B,N,N,2D+2]
    h   = inp @ W.T + b                              # [B,N,N,OUT]
    out = relu(sum_{j != i} h[:, :, j, :])           # [B,N,OUT]
    returns (diff, out)

By linearity of the edge Linear over the concat, with W = [W_i | W_j | W_d]
(cols 0:D, D:2D, 2D:2D+2) and Atilde = A with zeroed diagonal:

    out[b,i] = relu( (N-1) * (W_i @ af[b,i] + b)
                   + W_j @ (Atilde[b] @ af[b])[i]
                   + W_d @ (sum_j Atilde[b,i,j] * diff[b,i,j,:]) )

so the [B,N,N,2D+2] edge tensor never needs to be materialized.

Sharding: data-parallel over B across 8 NeuronCores (16 batches/core).

Device layout is fully transposed (out index o on partitions, (b,i) on the
free dim) so that:
  - stage 1, per batch: one matmul with af[b] ([j,d], natural layout)
    stationary and [Atilde[b]^T | 63*I] moving produces [h^T | 63*af^T]
    (h = Atilde @ af) in a single PSUM tile.
  - stage 2, batched across 8 batches per matmul (512 moving cols):
    W_j^T / W_i^T / W_d-expanded stationary, accumulating into one PSUM
    bank; the diff term's moving operand is (Atilde expanded over k) * diff^T,
    one vector multiply.
  - epilogue: relu + per-partition bias (63*b) in one ACT instruction.
"""

import sys

for _p in ("/opt/trn_rl_repo",):
    if _p not in sys.path:
        sys.path.insert(0, _p)

import numpy as np

B, N, D, OUT = 128, 64, 128, 128
NCORES = 8
BL = B // NCORES  # batches per core

# matmul dtype knobs: "float32" (4 cyc/row) or "float32r" (1 cyc/row at
# >=256 moving cols; fp32 bit layout, PE replicated-fp32 mode)
MM_STAGE1 = "float32"
MM_STAGE2 = "float32"

_cache = {}


def _build_nc():
    """Build (once) the single-core Bass/Tile program; all 8 cores run it
    SPMD on their own batch shard."""
    key = (MM_STAGE1, MM_STAGE2)
    if key in _cache:
        return _cache[key]

    from contextlib import ExitStack

    import concourse.bass as bass
    import concourse.mybir as mybir
    import concourse.tile as tile
    from concourse import bacc

    f32 = mybir.dt.float32
    dt1 = getattr(mybir.dt, MM_STAGE1)
    dt2 = getattr(mybir.dt, MM_STAGE2)

    nc = bacc.Bacc("TRN2", target_bir_lowering=False, debug=False, num_devices=NCORES)

    # DRAM I/O (per-core shapes)
    af_all = nc.dram_tensor("af_all", [N, BL * D], f32, kind="ExternalInput")
    ati_all = nc.dram_tensor("ati_all", [N, BL * 2 * N], f32, kind="ExternalInput")
    a2t = nc.dram_tensor("a2t", [2 * N, BL * N], f32, kind="ExternalInput")
    difft = nc.dram_tensor("difft", [2 * N, BL * N], f32, kind="ExternalInput")
    wjt = nc.dram_tensor("wjt", [D, OUT], f32, kind="ExternalInput")
    wit = nc.dram_tensor("wit", [D, OUT], f32, kind="ExternalInput")
    wdbig = nc.dram_tensor("wdbig", [2 * N, OUT], f32, kind="ExternalInput")
    bias63 = nc.dram_tensor("bias63", [OUT, 1], f32, kind="ExternalInput")
    outT = nc.dram_tensor("outT", [OUT, BL * N], f32, kind="ExternalOutput")

    H = BL // 2 * N  # 512: free-dim half (8 batches)

    with tile.TileContext(nc) as tc, ExitStack() as ctx:
        consts = ctx.enter_context(tc.tile_pool(name="consts", bufs=1))
        big = ctx.enter_context(tc.tile_pool(name="big", bufs=1))
        hx_pool = ctx.enter_context(tc.tile_pool(name="hx_ps", bufs=4, space="PSUM"))
        u_pool = ctx.enter_context(tc.tile_pool(name="u_ps", bufs=2, space="PSUM"))

        af_sb = big.tile([N, BL * D], f32)
        ati_sb = big.tile([N, BL * 2 * N], f32)
        a2t_sb = big.tile([2 * N, BL * N], f32)
        difft_sb = big.tile([2 * N, BL * N], f32)
        wd_sb = big.tile([2 * N, BL * N], f32)
        hx_sb = big.tile([D, BL * 2 * N], f32)
        outT_sb = big.tile([OUT, BL * N], f32)

        wjt_sb = consts.tile([D, OUT], f32)
        wit_sb = consts.tile([D, OUT], f32)
        wdbig_sb = consts.tile([2 * N, OUT], f32)
        bias_sb = consts.tile([OUT, 1], f32)

        # first chunk of stage-1 inputs, then constants, then the rest
        nc.sync.dma_start(af_sb[:, 0 : 4 * D], af_all[:, 0 : 4 * D])
        nc.sync.dma_start(ati_sb[:, 0 : 4 * 2 * N], ati_all[:, 0 : 4 * 2 * N])
        nc.sync.dma_start(wjt_sb[:], wjt[:])
        nc.sync.dma_start(wit_sb[:], wit[:])
        nc.sync.dma_start(wdbig_sb[:], wdbig[:])
        nc.sync.dma_start(bias_sb[:], bias63[:])
        for q in range(1, 4):
            s = slice(q * 4 * D, (q + 1) * 4 * D)
            nc.sync.dma_start(af_sb[:, s], af_all[:, s])
            nc.sync.dma_start(ati_sb[:, s], ati_all[:, s])
        for h in range(2):
            s = slice(h * H, (h + 1) * H)
            nc.sync.dma_start(a2t_sb[:, s], a2t[:, s])
            nc.sync.dma_start(difft_sb[:, s], difft[:, s])

        hx3 = hx_sb[:].rearrange("p (b c) -> p b c", c=2 * N)

        def mm_ap(t, d):
            return t if d == f32 else t.bitcast(d)

        for h in range(2):
            # the elementwise product feeding this half's diff-term matmul
            s = slice(h * H, (h + 1) * H)
            nc.vector.tensor_mul(wd_sb[:, s], a2t_sb[:, s], difft_sb[:, s])

            # stage 1: per-batch [h^T | 63 af^T] = (af[b]).T-free matmul
            for b in range(h * (BL // 2), (h + 1) * (BL // 2)):
                hx_ps = hx_pool.tile([D, 2 * N], f32)
                cs = slice(b * 2 * N, (b + 1) * 2 * N)
                nc.tensor.matmul(
                    hx_ps[:],
                    mm_ap(af_sb[:, b * D : (b + 1) * D], dt1),
                    mm_ap(ati_sb[:, cs], dt1),
                    start=True,
                    stop=True,
                )
                if b % 2 == 0:
                    nc.scalar.copy(hx_sb[:, cs], hx_ps[:])
                else:
                    nc.vector.tensor_copy(hx_sb[:, cs], hx_ps[:])

            # stage 2: batched over this half's 8 batches (512 moving cols)
            bs = slice(h * (BL // 2), (h + 1) * (BL // 2))
            u_ps = u_pool.tile([OUT, H], f32)
            nc.tensor.matmul(
                u_ps[:],
                mm_ap(wjt_sb[:], dt2),
                mm_ap(hx3[:, bs, 0:N], dt2),
                start=True,
                stop=False,
            )
            nc.tensor.matmul(
                u_ps[:],
                mm_ap(wit_sb[:], dt2),
                mm_ap(hx3[:, bs, N : 2 * N], dt2),
                start=False,
                stop=False,
            )
            nc.tensor.matmul(
                u_ps[:],
                mm_ap(wdbig_sb[:], dt2),
                mm_ap(wd_sb[:, s], dt2),
                start=False,
                stop=True,
            )
            nc.scalar.activation(
                outT_sb[:, s],
                u_ps[:],
                mybir.ActivationFunctionType.Relu,
                bias=bias_sb[:],
                scale=1.0,
            )
            nc.sync.dma_start(outT[:, s], outT_sb[:, s])

    nc.compile()
    _cache[key] = nc
    return nc


def _prep_in_maps(diff_vecs, af, A, W, bvec):
    """Host-side shard + relayout. Returns list of per-core input dicts."""
    eye = np.eye(N, dtype=np.float32)
    At = A * (1.0 - eye)[None]  # zero the diagonal: j == i excluded

    wjt = np.ascontiguousarray(W[:, D : 2 * D].T)
    wit = np.ascontiguousarray(W[:, 0:D].T)
    wdbig = np.ascontiguousarray(np.tile(W[:, 2 * D : 2 * D + 2].T, (N, 1)))
    bias63 = np.ascontiguousarray(((N - 1.0) * bvec).reshape(OUT, 1).astype(np.float32))
    ati_eye = np.broadcast_to(
        ((N - 1.0) * eye)[:, None, :], (N, BL, N)
    )  # [j, b, i] = 63*delta_ij

    in_maps = []
    for c in range(NCORES):
        sl = slice(c * BL, (c + 1) * BL)
        af_l = af[sl]  # [BL, N, D]
        At_l = At[sl]  # [BL, i, j]
        diff_l = diff_vecs[sl]  # [BL, i, j, 2]
        At_jbi = At_l.transpose(2, 0, 1)  # [j, b, i]
        in_maps.append(
            {
                "af_all": np.ascontiguousarray(
                    af_l.transpose(1, 0, 2).reshape(N, BL * D)
                ),
                "ati_all": np.ascontiguousarray(
                    np.concatenate([At_jbi, ati_eye], axis=2).reshape(N, BL * 2 * N)
                ),
                "a2t": np.ascontiguousarray(
                    np.repeat(At_jbi, 2, axis=0).reshape(2 * N, BL * N)
                ),
                "difft": np.ascontiguousarray(
                    diff_l.transpose(2, 3, 0, 1).reshape(2 * N, BL * N)
                ),
                "wjt": wjt,
                "wit": wit,
                "wdbig": wdbig,
                "bias63": bias63,
            }
        )
    return in_maps


def _gather(results):
    """[8] x outT[OUT, BL*N] -> out[B, N, OUT]"""
    outT = np.stack([results[c]["outT"] for c in range(NCORES)], axis=0)
    return np.ascontiguousarray(
        outT.reshape(NCORES, OUT, BL, N).transpose(0, 2, 3, 1).reshape(B, N, OUT)
    )


def kernel(**inputs):
    from concourse.bass_utils import run_bass_kernel_spmd

    diff_vecs = np.ascontiguousarray(np.asarray(inputs["diff_vecs"], dtype=np.float32))
    af = np.asarray(inputs["agent_features"], dtype=np.float32)
    A = np.asarray(inputs["A"], dtype=np.float32)
    W = np.asarray(inputs["W"], dtype=np.float32)
    bvec = np.asarray(inputs["b"], dtype=np.float32).reshape(-1)

    nc = _build_nc()
    in_maps = _prep_in_maps(diff_vecs, af, A, W, bvec)
    res = run_bass_kernel_spmd(nc, in_maps, list(range(NCORES))).results
    return diff_vecs, _gather(res)
